# revision 36
# baseline (speedup 1.0000x reference)
# CATS-SwiGLU decode kernel for TRN2 (8 NeuronCores, SPMD tensor-parallel).
#
# Reference computation (decode path, B=S=1):
#   x1    = silu(x @ Wgatet)                  [1,1,dff]
#   flags = |x1| > threshold
#   z     = where(flags, (x @ Wup.T) * x1, 0) [1,1,dff]
#   out   = z @ Wdownt                        [1,1,d]
#
# Sharding: d_ff (11008) split across 8 cores (1376 rows each). Each core
# computes its z slice and a full-width partial down-projection; the host
# sums the 8 partials (the all-reduce of the TP hint, done on host).
#
# All weights are fp16 on device (host cast; measured end-to-end rel err
# 2.4e-3 vs the 2e-2 gate), halving HBM traffic vs fp32. The kernel is
# then jointly limited by the two GEMV consumers, so the work is split:
#   - DVE (affine_mul_reduce, ~227 GB/s fp16): gate chunks 0-10 and up
#     chunks 0-5, rows-on-partitions layout, plus the CATS masking math.
#   - PE: up rows 768-1376 in row form (x k-columns stationary, transposed
#     Wup k-tiles moving, accumulating a [1,608] PSUM row at partition 32
#     of banks 0/1), and the whole down-projection (zm16 column stationary,
#     natural-layout Wdownt rows moving, 8x512 PSUM groups at partition 0
#     across the 11 f-tiles).
# The PE's u row is cast to fp16 and flipped into per-chunk columns with
# tiny rearranged SBUF->SBUF DMAs, all off the critical path (ready before
# the gate stream delivers x1 for those chunks).
# DMA rings: sync carries wg; gpsimd carries wu/wpe/wd; scalar carries the
# constants and stays free for the ACT masking ops, so every ring's queue
# issues back-to-back. ~35MB/core total HBM traffic.
import sys

for _p in ("/opt/trn_rl_repo",):
    if _p not in sys.path:
        sys.path.insert(0, _p)

import numpy as np

import concourse.bass as bass
import concourse.tile as tile
from concourse import bacc, mybir
from concourse.bass_utils import run_bass_kernel_spmd

D = 4096
FF = 11008
NCORES = 8
FSH = FF // NCORES          # 1376 rows of d_ff per core
NCH = (FSH + 127) // 128    # 11 chunks of <=128 rows
LAST = FSH - 128 * (NCH - 1)  # 96 rows in the last chunk
NB = 8                      # down-proj PSUM banks (512 d-columns each)
NDVE = 6                    # up chunks 0..NDVE-1 on DVE; rest on PE
PER = FSH - 128 * NDVE      # 608 dff rows of up handled by the PE
KT = D // 128               # 32 k-tiles for the PE up row-form
# wpe k-tiles fetched per main-loop chunk (front-loaded over chunks 0..5)
WPE_SCHED = (6, 6, 6, 6, 4, 4) + (0,) * (NCH - 6)
F16 = mybir.dt.float16
F32 = mybir.dt.float32

_CACHE = {}


def _bcast(ap, parts):
    """Replicate a 1-D AP across `parts` partitions (0-stride partition dim)."""
    return bass.AP(tensor=ap.tensor, offset=ap.offset, ap=[[0, parts]] + list(ap.ap))


def _build_nc():
    nc = bacc.Bacc("TRN2", target_bir_lowering=False, debug=False)

    x_d = nc.dram_tensor("x", [D], F16, kind="ExternalInput")
    xc_d = nc.dram_tensor("xc", [128, KT], F16, kind="ExternalInput")
    wg_d = nc.dram_tensor("wg", [FSH, D], F16, kind="ExternalInput")
    wu_d = nc.dram_tensor("wu", [NDVE * 128, D], F16, kind="ExternalInput")
    wup_d = nc.dram_tensor("wup", [KT, 128, PER], F16, kind="ExternalInput")
    wd_d = nc.dram_tensor("wd", [FSH, D], F16, kind="ExternalInput")
    thr_d = nc.dram_tensor("thr", [1], F32, kind="ExternalInput")
    out_d = nc.dram_tensor("out", [D], F32, kind="ExternalOutput")
    # DRAM bounce buffer for the u row->column flip (SBUF partition steps
    # can't express the transpose; DRAM strides can)
    scr_d = nc.dram_tensor("urow_scr", [(NCH - NDVE) * 128], F16, kind="Internal")

    with tile.TileContext(nc) as tc:
        with (
            tc.tile_pool(name="const", bufs=1) as const_pool,
            tc.tile_pool(name="wgpool", bufs=4) as wgpool,
            tc.tile_pool(name="wupool", bufs=4) as wupool,
            tc.tile_pool(name="wpepool", bufs=8) as wpepool,
            tc.tile_pool(name="wdpool", bufs=4) as wdpool,
            tc.tile_pool(name="acts", bufs=1) as acts,
            tc.tile_pool(name="psum", bufs=1, space="PSUM") as psum,
        ):
            # xc first (tiny, unblocks the PE); the x broadcast is split into
            # four 32-partition DMAs across rings — a single 128-partition
            # broadcast serializes its 8KB packet writes and takes ~15us,
            # which gated the first gate GEMV at 18.4us in the trace
            xc = const_pool.tile([128, KT], F16)
            nc.scalar.dma_start(out=xc[:], in_=xc_d.ap())
            x_rep = const_pool.tile([128, D], F16)
            nc.scalar.dma_start(out=x_rep[0:32, :], in_=_bcast(x_d.ap(), 32))
            nc.sync.dma_start(out=x_rep[32:64, :], in_=_bcast(x_d.ap(), 32))
            nc.gpsimd.dma_start(out=x_rep[64:96, :], in_=_bcast(x_d.ap(), 32))
            nc.scalar.dma_start(out=x_rep[96:128, :], in_=_bcast(x_d.ap(), 32))
            thr_sb = const_pool.tile([128, 1], F32)
            nc.scalar.dma_start(out=thr_sb[:], in_=_bcast(thr_d.ap(), 128))

            # warm the sigmoid/abs ACT tables while the DMA streams run
            warm = acts.tile([128, 1], F32)
            nc.scalar.activation(
                warm[:], thr_sb[:], mybir.ActivationFunctionType.Sigmoid
            )
            nc.scalar.activation(
                warm[:], thr_sb[:], mybir.ActivationFunctionType.Abs
            )

            x1 = acts.tile([128, NCH], F32)   # gate pre-activation
            u = acts.tile([128, NDVE], F32)   # up projection (DVE chunks)
            sg = acts.tile([128, NCH], F32)   # sigmoid(x1)
            x1s = acts.tile([128, NCH], F32)  # silu(x1)
            absx = acts.tile([128, NCH], F32)
            mask = acts.tile([128, NCH], F32)
            z = acts.tile([128, NCH], F32)
            zm16 = acts.tile([128, NCH], F16)  # masked z, fp16 for the PE
            u_row = const_pool.tile([1, PER], F16)   # PE up result, row form
            u_col = const_pool.tile([128, NCH - NDVE], F16)  # transposed
            osb = const_pool.tile([1, D], F32)

            # PSUM: 8 full banks; down groups use partition 0, the PE up row
            # accumulates at partition 32 of banks 0/1 (disjoint bytes)
            banks = [
                psum.tile([128, 512], F32, name=f"bank{n}") for n in range(NB)
            ]

            def up_pe_ktile(k):
                """One k-tile of the PE up row: u_row += x_k * WupT_k."""
                wpe = wpepool.tile([128, PER], F16, tag="wpe", name="wpe")
                nc.sync.dma_start(out=wpe[:], in_=wup_d.ap()[k])
                nc.tensor.matmul(
                    out=banks[0][32:33, 0:512],
                    lhsT=xc[:, k : k + 1],
                    rhs=wpe[:, 0:512],
                    start=(k == 0),
                    stop=(k == KT - 1),
                    skip_group_check=True,
                )
                nc.tensor.matmul(
                    out=banks[1][32:33, 0 : PER - 512],
                    lhsT=xc[:, k : k + 1],
                    rhs=wpe[:, 512:PER],
                    start=(k == 0),
                    stop=(k == KT - 1),
                    skip_group_check=True,
                )

            def mask_chunk(c, p, usrc, pre_z_hook=None):
                """CATS masking for chunk c -> zm16[:, c] (fp16)."""
                cc = slice(c, c + 1)
                nc.scalar.activation(
                    sg[:p, cc], x1[:p, cc], mybir.ActivationFunctionType.Sigmoid
                )
                nc.vector.tensor_mul(x1s[:p, cc], x1[:p, cc], sg[:p, cc])
                nc.scalar.activation(
                    absx[:p, cc], x1s[:p, cc], mybir.ActivationFunctionType.Abs
                )
                if pre_z_hook is not None:
                    pre_z_hook()
                nc.vector.tensor_scalar(
                    out=mask[:p, cc],
                    in0=absx[:p, cc],
                    scalar1=thr_sb[:p],
                    scalar2=None,
                    op0=mybir.AluOpType.is_gt,
                )
                nc.vector.tensor_mul(z[:p, cc], usrc, x1s[:p, cc])
                nc.vector.tensor_mul(zm16[:p, cc], z[:p, cc], mask[:p, cc])

            kt_next = 0
            for c in range(NCH):
                p = 128 if c < NCH - 1 else LAST
                rows = slice(c * 128, c * 128 + p)
                cc = slice(c, c + 1)

                # weight-stream DMAs first on each ring
                wgt = wgpool.tile([128, D], F16, tag="wg", name="wgt")
                nc.sync.dma_start(out=wgt[:p, :], in_=wg_d.ap()[rows, :])
                if c < NDVE:
                    wut = wupool.tile([128, D], F16, tag="wu", name="wut")
                    nc.gpsimd.dma_start(out=wut[:p, :], in_=wu_d.ap()[rows, :])
                wdt = wdpool.tile([128, D], F16, tag="wd", name="wdt")
                nc.gpsimd.dma_start(out=wdt[:p, :], in_=wd_d.ap()[rows, :])

                # PE up-projection k-tiles scheduled across early chunks
                for _ in range(WPE_SCHED[c]):
                    up_pe_ktile(kt_next)
                    kt_next += 1

                if c == NDVE:
                    # PE up row is complete: cast to fp16 and bounce through
                    # DRAM (write here, transposed read next chunk; the 1.2KB
                    # write finishes ~12us before the read issues)
                    nc.scalar.copy(u_row[0:1, 0:512], banks[0][32:33, 0:512])
                    nc.scalar.copy(
                        u_row[0:1, 512:PER], banks[1][32:33, 0 : PER - 512]
                    )
                    nc.scalar.dma_start(out=scr_d.ap()[0:PER], in_=u_row[0:1, :])

                def u_col_readback():
                    # transposed read-back: u_col[p, t] = u_row[t*128 + p].
                    # Issued on the scalar ring after this chunk's ACT ops
                    # (which wait on x1[c]), ~12us after the scr_d write.
                    nc.scalar.dma_start(
                        out=u_col[:, :],
                        in_=bass.AP(
                            tensor=scr_d,
                            offset=0,
                            ap=[[1, 128], [128, NCH - NDVE]],
                        ),
                    )

                # gate GEMV on DVE
                nc.vector.affine_mul_reduce(
                    out=wgt[:p, :],
                    accum_out=x1[:p, cc],
                    in0=wgt[:p, :],
                    in1=x_rep[:p, :],
                    scale=1.0,
                    bias=0.0,
                )
                if c < NDVE:
                    nc.vector.affine_mul_reduce(
                        out=wut[:p, :],
                        accum_out=u[:p, cc],
                        in0=wut[:p, :],
                        in1=x_rep[:p, :],
                        scale=1.0,
                        bias=0.0,
                    )
                    mask_chunk(c, p, u[:p, cc])
                else:
                    mask_chunk(
                        c,
                        p,
                        u_col[:p, c - NDVE : c - NDVE + 1],
                        pre_z_hook=u_col_readback if c == NDVE else None,
                    )

                # down projection on the PE: rank-1 contribution into PSUM
                for n in range(NB):
                    nc.tensor.matmul(
                        out=banks[n][0:1, :],
                        lhsT=zm16[:p, cc],
                        rhs=wdt[:p, n * 512 : (n + 1) * 512],
                        start=(c == 0),
                        stop=(c == NCH - 1),
                        skip_group_check=True,
                    )

            # drain PSUM -> SBUF on two engines in parallel, then DMA out
            for n in range(NB):
                half = slice(n * 512, (n + 1) * 512)
                if n < NB // 2:
                    nc.scalar.copy(osb[0:1, half], banks[n][0:1, :])
                else:
                    nc.vector.tensor_scalar_add(osb[0:1, half], banks[n][0:1, :], 0.0)
            nc.sync.dma_start(out=out_d.ap(), in_=osb[0:1, :])

    nc.compile()
    return nc


def _get_nc():
    if "nc" not in _CACHE:
        _CACHE["nc"] = _build_nc()
    return _CACHE["nc"]


def make_in_maps(x, Wup, Wgatet, Wdownt, threshold):
    """Shard full inputs into the 8 per-core input maps (fp16 weights)."""
    x16 = np.ascontiguousarray(
        np.asarray(x, dtype=np.float32).reshape(D).astype(np.float16)
    )
    xc = np.ascontiguousarray(x16.reshape(KT, 128).T)            # [128, KT]
    thr = np.asarray(threshold, dtype=np.float32).reshape(1)
    Wup = np.asarray(Wup, dtype=np.float32)
    Wgatet = np.asarray(Wgatet, dtype=np.float32)
    Wdownt = np.asarray(Wdownt, dtype=np.float32)
    in_maps = []
    for i in range(NCORES):
        sl = slice(i * FSH, (i + 1) * FSH)
        wg = np.ascontiguousarray(Wgatet[:, sl].T.astype(np.float16))  # [FSH, D]
        wu_full = Wup[sl, :].astype(np.float16)                        # [FSH, D]
        wu = np.ascontiguousarray(wu_full[: NDVE * 128, :])
        wup = np.ascontiguousarray(
            wu_full[NDVE * 128 :, :].T.reshape(KT, 128, PER)
        )
        wd = np.ascontiguousarray(Wdownt[sl, :].astype(np.float16))    # [FSH, D]
        in_maps.append(
            {"x": x16, "xc": xc, "wg": wg, "wu": wu, "wup": wup, "wd": wd,
             "thr": thr}
        )
    return in_maps


def run_sharded(x, Wup, Wgatet, Wdownt, threshold, trace=False, tmpdir=None):
    """Run on the 8 NeuronCores; returns (full_output, BassKernelResults)."""
    nc = _get_nc()
    in_maps = make_in_maps(x, Wup, Wgatet, Wdownt, threshold)
    res = run_bass_kernel_spmd(
        nc, in_maps, list(range(NCORES)), trace=trace, tmpdir=tmpdir
    )
    # un-shard: sum the 8 full-width partial down-projections
    acc = np.zeros(D, dtype=np.float64)
    for r in res.results:
        acc += r["out"].reshape(D).astype(np.float64)
    out = acc.astype(np.float32).reshape(1, 1, D)
    return out, res


def kernel(x, Wup, Wgatet, Wdownt, threshold):
    out, _ = run_sharded(x, Wup, Wgatet, Wdownt, threshold)
    return out


# revision 37
# speedup vs baseline: 1.1372x; 1.1372x over previous
# CATS-SwiGLU decode kernel for TRN2 (8 NeuronCores, SPMD tensor-parallel).
#
# Reference computation (decode path, B=S=1):
#   x1    = silu(x @ Wgatet)                  [1,1,dff]
#   flags = |x1| > threshold
#   z     = where(flags, (x @ Wup.T) * x1, 0) [1,1,dff]
#   out   = z @ Wdownt                        [1,1,d]
#
# Sharding: d_ff (11008) split across 8 cores (1376 rows each). Each core
# computes its z slice and a full-width partial down-projection; the host
# sums the 8 partials (the all-reduce of the TP hint, done on host).
#
# All weights are fp16 on device (host cast; measured end-to-end rel err
# 2.4e-3 vs the 2e-2 gate), halving HBM traffic vs fp32. The kernel is
# then jointly limited by the two GEMV consumers, so the work is split:
#   - DVE (affine_mul_reduce, ~227 GB/s fp16): gate chunks 0-10 and up
#     chunks 0-5, rows-on-partitions layout, plus the CATS masking math.
#   - PE: up rows 768-1376 in row form (x k-columns stationary, transposed
#     Wup k-tiles moving, accumulating a [1,608] PSUM row at partition 32
#     of banks 0/1), and the whole down-projection (zm16 column stationary,
#     natural-layout Wdownt rows moving, 8x512 PSUM groups at partition 0
#     across the 11 f-tiles).
# The PE's u row is cast to fp16 and flipped into per-chunk columns with
# tiny rearranged SBUF->SBUF DMAs, all off the critical path (ready before
# the gate stream delivers x1 for those chunks).
# DMA rings: sync carries wg; gpsimd carries wu/wpe/wd; scalar carries the
# constants and stays free for the ACT masking ops, so every ring's queue
# issues back-to-back. ~35MB/core total HBM traffic.
import sys

for _p in ("/opt/trn_rl_repo",):
    if _p not in sys.path:
        sys.path.insert(0, _p)

import numpy as np

import concourse.bass as bass
import concourse.tile as tile
from concourse import bacc, mybir
from concourse.bass_utils import run_bass_kernel_spmd

D = 4096
FF = 11008
NCORES = 8
FSH = FF // NCORES          # 1376 rows of d_ff per core
NCH = (FSH + 127) // 128    # 11 chunks of <=128 rows
LAST = FSH - 128 * (NCH - 1)  # 96 rows in the last chunk
NB = 8                      # down-proj PSUM banks (512 d-columns each)
NDVE = 6                    # up chunks 0..NDVE-1 on DVE; rest on PE
PER = FSH - 128 * NDVE      # 608 dff rows of up handled by the PE
KT = D // 128               # 32 k-tiles for the PE up row-form
# wpe k-tiles fetched per main-loop chunk (front-loaded over chunks 0..5)
WPE_SCHED = (6, 6, 6, 6, 4, 4) + (0,) * (NCH - 6)
F16 = mybir.dt.float16
F32 = mybir.dt.float32

_CACHE = {}


def _bcast(ap, parts):
    """Replicate a 1-D AP across `parts` partitions (0-stride partition dim)."""
    return bass.AP(tensor=ap.tensor, offset=ap.offset, ap=[[0, parts]] + list(ap.ap))


def _build_nc():
    nc = bacc.Bacc("TRN2", target_bir_lowering=False, debug=False)

    x_d = nc.dram_tensor("x", [D], F16, kind="ExternalInput")
    xc_d = nc.dram_tensor("xc", [128, KT], F16, kind="ExternalInput")
    wg_d = nc.dram_tensor("wg", [FSH, D], F16, kind="ExternalInput")
    wu_d = nc.dram_tensor("wu", [NDVE * 128, D], F16, kind="ExternalInput")
    wup_d = nc.dram_tensor("wup", [KT, 128, PER], F16, kind="ExternalInput")
    wd_d = nc.dram_tensor("wd", [FSH, D], F16, kind="ExternalInput")
    thr_d = nc.dram_tensor("thr", [1], F32, kind="ExternalInput")
    out_d = nc.dram_tensor("out", [D], F32, kind="ExternalOutput")
    # DRAM bounce buffer for the u row->column flip (SBUF partition steps
    # can't express the transpose; DRAM strides can)
    scr_d = nc.dram_tensor("urow_scr", [(NCH - NDVE) * 128], F16, kind="Internal")

    with tile.TileContext(nc) as tc:
        with (
            tc.tile_pool(name="const", bufs=1) as const_pool,
            tc.tile_pool(name="wgpool", bufs=4) as wgpool,
            tc.tile_pool(name="wupool", bufs=4) as wupool,
            tc.tile_pool(name="wpepool", bufs=8) as wpepool,
            tc.tile_pool(name="wdpool", bufs=4) as wdpool,
            tc.tile_pool(name="acts", bufs=1) as acts,
            tc.tile_pool(name="psum", bufs=1, space="PSUM") as psum,
        ):
            # constants on the scalar ring so the weight streams start at t=0
            # (broadcast DMAs serialize their queue: splitting the x_rep
            # broadcast onto the sync/gpsimd rings blocks the weight streams
            # behind it and REGRESSES — measured 154us vs 128us)
            x_rep = const_pool.tile([128, D], F16)
            nc.scalar.dma_start(out=x_rep[:], in_=_bcast(x_d.ap(), 128))
            xc = const_pool.tile([128, KT], F16)
            nc.scalar.dma_start(out=xc[:], in_=xc_d.ap())
            thr_sb = const_pool.tile([128, 1], F32)
            nc.scalar.dma_start(out=thr_sb[:], in_=_bcast(thr_d.ap(), 128))

            # warm the sigmoid/abs ACT tables while the DMA streams run
            warm = acts.tile([128, 1], F32)
            nc.scalar.activation(
                warm[:], thr_sb[:], mybir.ActivationFunctionType.Sigmoid
            )
            nc.scalar.activation(
                warm[:], thr_sb[:], mybir.ActivationFunctionType.Abs
            )

            x1 = acts.tile([128, NCH], F32)   # gate pre-activation
            u = acts.tile([128, NDVE], F32)   # up projection (DVE chunks)
            sg = acts.tile([128, NCH], F32)   # sigmoid(x1)
            x1s = acts.tile([128, NCH], F32)  # silu(x1)
            absx = acts.tile([128, NCH], F32)
            mask = acts.tile([128, NCH], F32)
            z = acts.tile([128, NCH], F32)
            zm16 = acts.tile([128, NCH], F16)  # masked z, fp16 for the PE
            u_row = const_pool.tile([1, PER], F16)   # PE up result, row form
            u_col = const_pool.tile([128, NCH - NDVE], F16)  # transposed
            osb = const_pool.tile([1, D], F32)

            # PSUM: 8 full banks; down groups use partition 0, the PE up row
            # accumulates at partition 32 of banks 0/1 (disjoint bytes)
            banks = [
                psum.tile([128, 512], F32, name=f"bank{n}") for n in range(NB)
            ]

            def up_pe_ktile(k):
                """One k-tile of the PE up row: u_row += x_k * WupT_k."""
                wpe = wpepool.tile([128, PER], F16, tag="wpe", name="wpe")
                nc.sync.dma_start(out=wpe[:], in_=wup_d.ap()[k])
                nc.tensor.matmul(
                    out=banks[0][32:33, 0:512],
                    lhsT=xc[:, k : k + 1],
                    rhs=wpe[:, 0:512],
                    start=(k == 0),
                    stop=(k == KT - 1),
                    skip_group_check=True,
                )
                nc.tensor.matmul(
                    out=banks[1][32:33, 0 : PER - 512],
                    lhsT=xc[:, k : k + 1],
                    rhs=wpe[:, 512:PER],
                    start=(k == 0),
                    stop=(k == KT - 1),
                    skip_group_check=True,
                )

            def mask_chunk(c, p, usrc, pre_z_hook=None):
                """CATS masking for chunk c -> zm16[:, c] (fp16)."""
                cc = slice(c, c + 1)
                nc.scalar.activation(
                    sg[:p, cc], x1[:p, cc], mybir.ActivationFunctionType.Sigmoid
                )
                nc.vector.tensor_mul(x1s[:p, cc], x1[:p, cc], sg[:p, cc])
                nc.scalar.activation(
                    absx[:p, cc], x1s[:p, cc], mybir.ActivationFunctionType.Abs
                )
                if pre_z_hook is not None:
                    pre_z_hook()
                nc.vector.tensor_scalar(
                    out=mask[:p, cc],
                    in0=absx[:p, cc],
                    scalar1=thr_sb[:p],
                    scalar2=None,
                    op0=mybir.AluOpType.is_gt,
                )
                nc.vector.tensor_mul(z[:p, cc], usrc, x1s[:p, cc])
                nc.vector.tensor_mul(zm16[:p, cc], z[:p, cc], mask[:p, cc])

            kt_next = 0
            for c in range(NCH):
                p = 128 if c < NCH - 1 else LAST
                rows = slice(c * 128, c * 128 + p)
                cc = slice(c, c + 1)

                # weight-stream DMAs first on each ring
                wgt = wgpool.tile([128, D], F16, tag="wg", name="wgt")
                nc.sync.dma_start(out=wgt[:p, :], in_=wg_d.ap()[rows, :])
                if c < NDVE:
                    wut = wupool.tile([128, D], F16, tag="wu", name="wut")
                    nc.gpsimd.dma_start(out=wut[:p, :], in_=wu_d.ap()[rows, :])
                wdt = wdpool.tile([128, D], F16, tag="wd", name="wdt")
                nc.gpsimd.dma_start(out=wdt[:p, :], in_=wd_d.ap()[rows, :])

                # PE up-projection k-tiles scheduled across early chunks
                for _ in range(WPE_SCHED[c]):
                    up_pe_ktile(kt_next)
                    kt_next += 1

                if c == NDVE:
                    # PE up row is complete: cast to fp16 and bounce through
                    # DRAM (write here, transposed read next chunk; the 1.2KB
                    # write finishes ~12us before the read issues)
                    nc.scalar.copy(u_row[0:1, 0:512], banks[0][32:33, 0:512])
                    nc.scalar.copy(
                        u_row[0:1, 512:PER], banks[1][32:33, 0 : PER - 512]
                    )
                    nc.scalar.dma_start(out=scr_d.ap()[0:PER], in_=u_row[0:1, :])

                def u_col_readback():
                    # transposed read-back: u_col[p, t] = u_row[t*128 + p].
                    # Issued on the scalar ring after this chunk's ACT ops
                    # (which wait on x1[c]), ~12us after the scr_d write.
                    nc.scalar.dma_start(
                        out=u_col[:, :],
                        in_=bass.AP(
                            tensor=scr_d,
                            offset=0,
                            ap=[[1, 128], [128, NCH - NDVE]],
                        ),
                    )

                # gate GEMV on DVE
                nc.vector.affine_mul_reduce(
                    out=wgt[:p, :],
                    accum_out=x1[:p, cc],
                    in0=wgt[:p, :],
                    in1=x_rep[:p, :],
                    scale=1.0,
                    bias=0.0,
                )
                if c < NDVE:
                    nc.vector.affine_mul_reduce(
                        out=wut[:p, :],
                        accum_out=u[:p, cc],
                        in0=wut[:p, :],
                        in1=x_rep[:p, :],
                        scale=1.0,
                        bias=0.0,
                    )
                    mask_chunk(c, p, u[:p, cc])
                else:
                    mask_chunk(
                        c,
                        p,
                        u_col[:p, c - NDVE : c - NDVE + 1],
                        pre_z_hook=u_col_readback if c == NDVE else None,
                    )

                # down projection on the PE: rank-1 contribution into PSUM
                for n in range(NB):
                    nc.tensor.matmul(
                        out=banks[n][0:1, :],
                        lhsT=zm16[:p, cc],
                        rhs=wdt[:p, n * 512 : (n + 1) * 512],
                        start=(c == 0),
                        stop=(c == NCH - 1),
                        skip_group_check=True,
                    )

            # drain PSUM -> SBUF on two engines in parallel, then DMA out
            for n in range(NB):
                half = slice(n * 512, (n + 1) * 512)
                if n < NB // 2:
                    nc.scalar.copy(osb[0:1, half], banks[n][0:1, :])
                else:
                    nc.vector.tensor_scalar_add(osb[0:1, half], banks[n][0:1, :], 0.0)
            nc.sync.dma_start(out=out_d.ap(), in_=osb[0:1, :])

    nc.compile()
    return nc


def _get_nc():
    if "nc" not in _CACHE:
        _CACHE["nc"] = _build_nc()
    return _CACHE["nc"]


def make_in_maps(x, Wup, Wgatet, Wdownt, threshold):
    """Shard full inputs into the 8 per-core input maps (fp16 weights)."""
    x16 = np.ascontiguousarray(
        np.asarray(x, dtype=np.float32).reshape(D).astype(np.float16)
    )
    xc = np.ascontiguousarray(x16.reshape(KT, 128).T)            # [128, KT]
    thr = np.asarray(threshold, dtype=np.float32).reshape(1)
    Wup = np.asarray(Wup, dtype=np.float32)
    Wgatet = np.asarray(Wgatet, dtype=np.float32)
    Wdownt = np.asarray(Wdownt, dtype=np.float32)
    in_maps = []
    for i in range(NCORES):
        sl = slice(i * FSH, (i + 1) * FSH)
        wg = np.ascontiguousarray(Wgatet[:, sl].T.astype(np.float16))  # [FSH, D]
        wu_full = Wup[sl, :].astype(np.float16)                        # [FSH, D]
        wu = np.ascontiguousarray(wu_full[: NDVE * 128, :])
        wup = np.ascontiguousarray(
            wu_full[NDVE * 128 :, :].T.reshape(KT, 128, PER)
        )
        wd = np.ascontiguousarray(Wdownt[sl, :].astype(np.float16))    # [FSH, D]
        in_maps.append(
            {"x": x16, "xc": xc, "wg": wg, "wu": wu, "wup": wup, "wd": wd,
             "thr": thr}
        )
    return in_maps


def run_sharded(x, Wup, Wgatet, Wdownt, threshold, trace=False, tmpdir=None):
    """Run on the 8 NeuronCores; returns (full_output, BassKernelResults)."""
    nc = _get_nc()
    in_maps = make_in_maps(x, Wup, Wgatet, Wdownt, threshold)
    res = run_bass_kernel_spmd(
        nc, in_maps, list(range(NCORES)), trace=trace, tmpdir=tmpdir
    )
    # un-shard: sum the 8 full-width partial down-projections
    acc = np.zeros(D, dtype=np.float64)
    for r in res.results:
        acc += r["out"].reshape(D).astype(np.float64)
    out = acc.astype(np.float32).reshape(1, 1, D)
    return out, res


def kernel(x, Wup, Wgatet, Wdownt, threshold):
    out, _ = run_sharded(x, Wup, Wgatet, Wdownt, threshold)
    return out


# revision 42
# speedup vs baseline: 1.1447x; 1.0066x over previous
# CATS-SwiGLU decode kernel for TRN2 (8 NeuronCores, SPMD tensor-parallel).
#
# Reference computation (decode path, B=S=1):
#   x1    = silu(x @ Wgatet)                  [1,1,dff]
#   flags = |x1| > threshold
#   z     = where(flags, (x @ Wup.T) * x1, 0) [1,1,dff]
#   out   = z @ Wdownt                        [1,1,d]
#
# Sharding: d_ff (11008) split across 8 cores (1376 rows each). Each core
# computes its z slice and a full-width partial down-projection; the host
# sums the 8 partials (the all-reduce of the TP hint, done on host).
#
# All weights are fp16 on device (host cast; measured end-to-end rel err
# 2.4e-3 vs the 2e-2 gate), halving HBM traffic vs fp32. The kernel is
# then jointly limited by the two GEMV consumers, so the work is split:
#   - DVE (affine_mul_reduce, ~227 GB/s fp16): gate chunks 0-10 and up
#     chunks 0-5, rows-on-partitions layout, plus the CATS masking math.
#   - PE: up rows 768-1376 in row form (x k-columns stationary, transposed
#     Wup k-tiles moving, accumulating a [1,608] PSUM row at partition 32
#     of banks 0/1), and the whole down-projection (zm16 column stationary,
#     natural-layout Wdownt rows moving, 8x512 PSUM groups at partition 0
#     across the 11 f-tiles).
# The PE's u row is cast to fp16 and flipped into per-chunk columns with
# tiny rearranged SBUF->SBUF DMAs, all off the critical path (ready before
# the gate stream delivers x1 for those chunks).
# DMA rings: sync carries wg; gpsimd carries wu/wpe/wd; scalar carries the
# constants and stays free for the ACT masking ops, so every ring's queue
# issues back-to-back. ~35MB/core total HBM traffic.
import sys

for _p in ("/opt/trn_rl_repo",):
    if _p not in sys.path:
        sys.path.insert(0, _p)

import numpy as np

import concourse.bass as bass
import concourse.tile as tile
from concourse import bacc, mybir
from concourse.bass_utils import run_bass_kernel_spmd

D = 4096
FF = 11008
NCORES = 8
FSH = FF // NCORES          # 1376 rows of d_ff per core
NCH = (FSH + 127) // 128    # 11 chunks of <=128 rows
LAST = FSH - 128 * (NCH - 1)  # 96 rows in the last chunk
NB = 8                      # down-proj PSUM banks (512 d-columns each)
NDVE = 5                    # up chunks 0..NDVE-1 on DVE; rest on PE
PER = FSH - 128 * NDVE      # 736 dff rows of up handled by the PE
KT = D // 128               # 32 k-tiles for the PE up row-form
# wpe k-tiles fetched per main-loop chunk (front-loaded over chunks 0..4)
WPE_SCHED = (7, 7, 6, 6, 6) + (0,) * (NCH - 5)
F16 = mybir.dt.float16
F32 = mybir.dt.float32

_CACHE = {}


def _bcast(ap, parts):
    """Replicate a 1-D AP across `parts` partitions (0-stride partition dim)."""
    return bass.AP(tensor=ap.tensor, offset=ap.offset, ap=[[0, parts]] + list(ap.ap))


def _build_nc():
    nc = bacc.Bacc("TRN2", target_bir_lowering=False, debug=False)

    x_d = nc.dram_tensor("x", [D], F16, kind="ExternalInput")
    xc_d = nc.dram_tensor("xc", [128, KT], F16, kind="ExternalInput")
    wg_d = nc.dram_tensor("wg", [FSH, D], F16, kind="ExternalInput")
    wu_d = nc.dram_tensor("wu", [NDVE * 128, D], F16, kind="ExternalInput")
    wup_d = nc.dram_tensor("wup", [KT, 128, PER], F16, kind="ExternalInput")
    wd_d = nc.dram_tensor("wd", [FSH, D], F16, kind="ExternalInput")
    thr_d = nc.dram_tensor("thr", [1], F32, kind="ExternalInput")
    out_d = nc.dram_tensor("out", [D], F32, kind="ExternalOutput")
    # DRAM bounce buffer for the u row->column flip (SBUF partition steps
    # can't express the transpose; DRAM strides can)
    scr_d = nc.dram_tensor("urow_scr", [(NCH - NDVE) * 128], F16, kind="Internal")

    with tile.TileContext(nc) as tc:
        with (
            tc.tile_pool(name="const", bufs=1) as const_pool,
            tc.tile_pool(name="wgpool", bufs=6) as wgpool,
            tc.tile_pool(name="wupool", bufs=6) as wupool,
            tc.tile_pool(name="wpepool", bufs=8) as wpepool,
            tc.tile_pool(name="wdpool", bufs=4) as wdpool,
            tc.tile_pool(name="acts", bufs=1) as acts,
            tc.tile_pool(name="psum", bufs=1, space="PSUM") as psum,
        ):
            # constants on the scalar ring so the weight streams start at t=0
            # (broadcast DMAs serialize their queue: splitting the x_rep
            # broadcast onto the sync/gpsimd rings blocks the weight streams
            # behind it and REGRESSES — measured 154us vs 128us)
            x_rep = const_pool.tile([128, D], F16)
            nc.scalar.dma_start(out=x_rep[:], in_=_bcast(x_d.ap(), 128))
            xc = const_pool.tile([128, KT], F16)
            nc.scalar.dma_start(out=xc[:], in_=xc_d.ap())
            thr_sb = const_pool.tile([128, 1], F32)
            nc.scalar.dma_start(out=thr_sb[:], in_=_bcast(thr_d.ap(), 128))

            # warm the sigmoid ACT table while the DMA streams run
            warm = acts.tile([128, 1], F32)
            nc.scalar.activation(
                warm[:], thr_sb[:], mybir.ActivationFunctionType.Sigmoid
            )
            # threshold^2: the mask compares silu(x1)^2 > thr^2 so the whole
            # compare chain stays on the DVE (no ACT abs round-trip)
            thr2 = acts.tile([128, 1], F32)
            nc.vector.tensor_mul(thr2[:], thr_sb[:], thr_sb[:])

            x1 = acts.tile([128, NCH], F32)   # gate pre-activation
            u = acts.tile([128, NDVE], F32)   # up projection (DVE chunks)
            sg = acts.tile([128, NCH], F32)   # sigmoid(x1)
            x1s = acts.tile([128, NCH], F32)  # silu(x1)
            sq = acts.tile([128, NCH], F32)   # silu(x1)^2
            mask = acts.tile([128, NCH], F32)
            z = acts.tile([128, NCH], F32)
            zm16 = acts.tile([128, NCH], F16)  # masked z, fp16 for the PE
            u_row = const_pool.tile([1, PER], F16)   # PE up result, row form
            u_col = const_pool.tile([128, NCH - NDVE], F16)  # transposed
            osb = const_pool.tile([1, D], F32)

            # PSUM: 8 full banks; down groups use partition 0, the PE up row
            # accumulates at partition 32 of banks 0/1 (disjoint bytes)
            banks = [
                psum.tile([128, 512], F32, name=f"bank{n}") for n in range(NB)
            ]

            def up_pe_ktile(k):
                """One k-tile of the PE up row: u_row += x_k * WupT_k."""
                wpe = wpepool.tile([128, PER], F16, tag="wpe", name="wpe")
                nc.sync.dma_start(out=wpe[:], in_=wup_d.ap()[k])
                nc.tensor.matmul(
                    out=banks[0][32:33, 0:512],
                    lhsT=xc[:, k : k + 1],
                    rhs=wpe[:, 0:512],
                    start=(k == 0),
                    stop=(k == KT - 1),
                    skip_group_check=True,
                )
                nc.tensor.matmul(
                    out=banks[1][32:33, 0 : PER - 512],
                    lhsT=xc[:, k : k + 1],
                    rhs=wpe[:, 512:PER],
                    start=(k == 0),
                    stop=(k == KT - 1),
                    skip_group_check=True,
                )

            def mask_chunk(c, p, usrc, pre_z_hook=None):
                """CATS masking for chunk c -> zm16[:, c] (fp16)."""
                cc = slice(c, c + 1)
                nc.scalar.activation(
                    sg[:p, cc], x1[:p, cc], mybir.ActivationFunctionType.Sigmoid
                )
                if pre_z_hook is not None:
                    pre_z_hook()
                nc.vector.tensor_mul(x1s[:p, cc], x1[:p, cc], sg[:p, cc])
                nc.vector.tensor_mul(sq[:p, cc], x1s[:p, cc], x1s[:p, cc])
                nc.vector.tensor_scalar(
                    out=mask[:p, cc],
                    in0=sq[:p, cc],
                    scalar1=thr2[:p],
                    scalar2=None,
                    op0=mybir.AluOpType.is_gt,
                )
                nc.vector.tensor_mul(z[:p, cc], usrc, x1s[:p, cc])
                nc.vector.tensor_mul(zm16[:p, cc], z[:p, cc], mask[:p, cc])

            kt_next = 0
            for c in range(NCH):
                p = 128 if c < NCH - 1 else LAST
                rows = slice(c * 128, c * 128 + p)
                cc = slice(c, c + 1)

                # weight-stream DMAs first on each ring
                wgt = wgpool.tile([128, D], F16, tag="wg", name="wgt")
                nc.sync.dma_start(out=wgt[:p, :], in_=wg_d.ap()[rows, :])
                if c < NDVE:
                    wut = wupool.tile([128, D], F16, tag="wu", name="wut")
                    nc.gpsimd.dma_start(out=wut[:p, :], in_=wu_d.ap()[rows, :])
                wdt = wdpool.tile([128, D], F16, tag="wd", name="wdt")
                nc.gpsimd.dma_start(out=wdt[:p, :], in_=wd_d.ap()[rows, :])

                # PE up-projection k-tiles scheduled across early chunks
                for _ in range(WPE_SCHED[c]):
                    up_pe_ktile(kt_next)
                    kt_next += 1

                if c == NDVE:
                    # PE up row is complete: cast to fp16 and bounce through
                    # DRAM (write here, transposed read next chunk; the 1.2KB
                    # write finishes ~12us before the read issues)
                    nc.scalar.copy(u_row[0:1, 0:512], banks[0][32:33, 0:512])
                    nc.scalar.copy(
                        u_row[0:1, 512:PER], banks[1][32:33, 0 : PER - 512]
                    )
                    nc.scalar.dma_start(out=scr_d.ap()[0:PER], in_=u_row[0:1, :])

                def u_col_readback():
                    # transposed read-back: u_col[p, t] = u_row[t*128 + p].
                    # Issued on the scalar ring after this chunk's ACT ops
                    # (which wait on x1[c]), ~12us after the scr_d write.
                    nc.scalar.dma_start(
                        out=u_col[:, :],
                        in_=bass.AP(
                            tensor=scr_d,
                            offset=0,
                            ap=[[1, 128], [128, NCH - NDVE]],
                        ),
                    )

                # gate GEMV on DVE
                nc.vector.affine_mul_reduce(
                    out=wgt[:p, :],
                    accum_out=x1[:p, cc],
                    in0=wgt[:p, :],
                    in1=x_rep[:p, :],
                    scale=1.0,
                    bias=0.0,
                )
                if c < NDVE:
                    nc.vector.affine_mul_reduce(
                        out=wut[:p, :],
                        accum_out=u[:p, cc],
                        in0=wut[:p, :],
                        in1=x_rep[:p, :],
                        scale=1.0,
                        bias=0.0,
                    )
                    mask_chunk(c, p, u[:p, cc])
                else:
                    mask_chunk(
                        c,
                        p,
                        u_col[:p, c - NDVE : c - NDVE + 1],
                        pre_z_hook=u_col_readback if c == NDVE else None,
                    )

                # down projection on the PE: rank-1 contribution into PSUM
                for n in range(NB):
                    nc.tensor.matmul(
                        out=banks[n][0:1, :],
                        lhsT=zm16[:p, cc],
                        rhs=wdt[:p, n * 512 : (n + 1) * 512],
                        start=(c == 0),
                        stop=(c == NCH - 1),
                        skip_group_check=True,
                    )

            # drain PSUM -> SBUF on two engines in parallel, then DMA out
            for n in range(NB):
                half = slice(n * 512, (n + 1) * 512)
                if n < NB // 2:
                    nc.scalar.copy(osb[0:1, half], banks[n][0:1, :])
                else:
                    nc.vector.tensor_scalar_add(osb[0:1, half], banks[n][0:1, :], 0.0)
            nc.sync.dma_start(out=out_d.ap(), in_=osb[0:1, :])

    nc.compile()
    return nc


def _get_nc():
    if "nc" not in _CACHE:
        _CACHE["nc"] = _build_nc()
    return _CACHE["nc"]


def make_in_maps(x, Wup, Wgatet, Wdownt, threshold):
    """Shard full inputs into the 8 per-core input maps (fp16 weights)."""
    x16 = np.ascontiguousarray(
        np.asarray(x, dtype=np.float32).reshape(D).astype(np.float16)
    )
    xc = np.ascontiguousarray(x16.reshape(KT, 128).T)            # [128, KT]
    thr = np.asarray(threshold, dtype=np.float32).reshape(1)
    Wup = np.asarray(Wup, dtype=np.float32)
    Wgatet = np.asarray(Wgatet, dtype=np.float32)
    Wdownt = np.asarray(Wdownt, dtype=np.float32)
    in_maps = []
    for i in range(NCORES):
        sl = slice(i * FSH, (i + 1) * FSH)
        wg = np.ascontiguousarray(Wgatet[:, sl].T.astype(np.float16))  # [FSH, D]
        wu_full = Wup[sl, :].astype(np.float16)                        # [FSH, D]
        wu = np.ascontiguousarray(wu_full[: NDVE * 128, :])
        wup = np.ascontiguousarray(
            wu_full[NDVE * 128 :, :].T.reshape(KT, 128, PER)
        )
        wd = np.ascontiguousarray(Wdownt[sl, :].astype(np.float16))    # [FSH, D]
        in_maps.append(
            {"x": x16, "xc": xc, "wg": wg, "wu": wu, "wup": wup, "wd": wd,
             "thr": thr}
        )
    return in_maps


def run_sharded(x, Wup, Wgatet, Wdownt, threshold, trace=False, tmpdir=None):
    """Run on the 8 NeuronCores; returns (full_output, BassKernelResults)."""
    nc = _get_nc()
    in_maps = make_in_maps(x, Wup, Wgatet, Wdownt, threshold)
    res = run_bass_kernel_spmd(
        nc, in_maps, list(range(NCORES)), trace=trace, tmpdir=tmpdir
    )
    # un-shard: sum the 8 full-width partial down-projections
    acc = np.zeros(D, dtype=np.float64)
    for r in res.results:
        acc += r["out"].reshape(D).astype(np.float64)
    out = acc.astype(np.float32).reshape(1, 1, D)
    return out, res


def kernel(x, Wup, Wgatet, Wdownt, threshold):
    out, _ = run_sharded(x, Wup, Wgatet, Wdownt, threshold)
    return out
